# revision 6
# baseline (speedup 1.0000x reference)
"""Trainium2 Bass kernel for nn_Div_86887188398977.

Computes, per (batch, channel) image with C == 1:
    xp = pad(x[..., :-1], width (1,1));  yp = pad(y[..., :-1, :], height (1,1))
    out = kx0*xp[..., :-1] + kx1*xp[..., 1:] + ky0*yp[..., :-1, :] + ky1*yp[..., 1:, :]
i.e. per element (j = width, i = height):
    out[i, j] = kx0*x[i, j-1]         (j >= 1)
              + kx1*x[i, j]           (j <= W-2)
              + ky0*y[i-1, j]         (i >= 1)
              + ky1*y[i, j]           (i <= H-2)

Sharding: pure data parallel over the batch axis, 16 batches -> 8 cores x 2.

Per-core layout: images flattened to [4096, 2048] rows; row tiles of <=127
output rows with H (rows) on SBUF partitions and W contiguous on the free axis.

Work split per tile (all fp32, numerically exact):
  - dy (the cross-partition height shift) runs on the TensorEngine as 4
    accumulating fp32 matmuls (one per 512-col PSUM bank) with a banded
    lhsT that also encodes the height boundary masks:
       interior tiles: yt = y[r0-1 : r0+L]  (K = L+1), lhsT[m,m] = ky0,
                       lhsT[m+1,m] = ky1
       batch-first:    yt = y[r0 : r0+L]    (K = L),   lhsT[m,m] = ky1,
                       lhsT[m-1,m] = ky0    (row 0 drops the ky0 term)
       batch-last:     interior form with the ky1 coeff of the final row
                       zeroed (row H-1 drops the ky1 term)
  - dx mid columns (1..W-2) on GPSIMD (tensor_tensor, kx = +-1 fast path)
  - dx edge columns 0 / W-1 on ScalarE
  - final out = dx + dy on the VectorEngine (one TT add, PSUM operand)
  - loads on the SP HWDGE ring, stores on the ACT HWDGE ring

For general (non +-1) kx the dx term is emitted as additional fp32 PE
matmuls with scaled-identity weights instead (slower but exact).
"""

import sys

if "/opt/trn_rl_repo" not in sys.path:
    sys.path.insert(0, "/opt/trn_rl_repo")

import numpy as np

import concourse.bacc as bacc
import concourse.mybir as mybir
from concourse.mybir import AluOpType
from concourse.tile import TileContext
from concourse.bass_utils import run_bass_kernel_spmd

B, C, H, W = 16, 1, 2048, 2048
NCORES = 8
BPC = B // NCORES  # batches per core
RPC = BPC * H  # flattened rows per core
F32 = mybir.dt.float32
LMAX = 127
NBANK = W // 512


def _batch_tiles():
    """(r0, L, kind) within one H=2048 image."""
    tiles = []
    r0 = 0
    while r0 < H:
        L = min(LMAX, H - r0)
        kind = "first" if r0 == 0 else ("last" if r0 + L == H else "int")
        tiles.append((r0, L, kind))
        r0 += L
    return tiles


def _weights(kx, ky):
    ky0, ky1 = ky
    kx0, kx1 = kx
    L = LMAX
    last_L = _batch_tiles()[-1][1]
    wy_first = np.zeros((L, L), dtype=np.float32)
    wy_first[np.arange(L), np.arange(L)] = ky1
    wy_first[np.arange(L - 1), np.arange(L - 1) + 1] = ky0
    wy_int = np.zeros((L + 1, L), dtype=np.float32)
    wy_int[np.arange(L), np.arange(L)] = ky0
    wy_int[np.arange(L) + 1, np.arange(L)] = ky1
    wy_last = np.zeros((last_L + 1, last_L), dtype=np.float32)
    wy_last[np.arange(last_L), np.arange(last_L)] = ky0
    wy_last[np.arange(last_L) + 1, np.arange(last_L)] = ky1
    wy_last[last_L, last_L - 1] = 0.0
    wx0 = kx0 * np.eye(L, dtype=np.float32)
    wx1 = kx1 * np.eye(L, dtype=np.float32)
    return {
        "wy_first": wy_first,
        "wy_int": wy_int,
        "wy_last": wy_last,
        "wx0": wx0,
        "wx1": wx1,
    }


def _build(kx, ky, repeat=1):
    fast_dx = kx in ((-1.0, 1.0), (1.0, -1.0))
    last_L = _batch_tiles()[-1][1]

    nc = bacc.Bacc("TRN2", target_bir_lowering=False, debug=False, num_devices=NCORES)
    x_d = nc.declare_dram_parameter("x", [RPC, W], F32, isOutput=False)
    y_d = nc.declare_dram_parameter("y", [RPC, W], F32, isOutput=False)
    wyf_d = nc.declare_dram_parameter("wy_first", [LMAX, LMAX], F32, isOutput=False)
    wyi_d = nc.declare_dram_parameter("wy_int", [LMAX + 1, LMAX], F32, isOutput=False)
    wyl_d = nc.declare_dram_parameter("wy_last", [last_L + 1, last_L], F32, isOutput=False)
    wx0_d = nc.declare_dram_parameter("wx0", [LMAX, LMAX], F32, isOutput=False)
    wx1_d = nc.declare_dram_parameter("wx1", [LMAX, LMAX], F32, isOutput=False)
    out_d = nc.declare_dram_parameter("out", [RPC, W], F32, isOutput=True)

    with TileContext(nc) as tc:
        with (
            tc.tile_pool(name="wpool", bufs=1) as wpool,
            tc.tile_pool(name="io", bufs=4) as io,
            tc.tile_pool(name="ps", bufs=2, space="PSUM") as ps,
        ):
            wyf = wpool.tile([LMAX, LMAX], F32)
            nc.sync.dma_start(wyf[:], wyf_d[:])
            wyi = wpool.tile([LMAX + 1, LMAX], F32)
            nc.sync.dma_start(wyi[:], wyi_d[:])
            wyl = wpool.tile([last_L + 1, last_L], F32)
            nc.sync.dma_start(wyl[:], wyl_d[:])
            wx0 = wpool.tile([LMAX, LMAX], F32)
            nc.sync.dma_start(wx0[:], wx0_d[:])
            wx1 = wpool.tile([LMAX, LMAX], F32)
            nc.sync.dma_start(wx1[:], wx1_d[:])

            for b in range(repeat * BPC):
                base = (b % BPC) * H
                for r0l, L, kind in _batch_tiles():
                    r0 = base + r0l
                    xt = io.tile([LMAX, W], F32, tag="xt", name="xt")
                    nc.sync.dma_start(xt[0:L, :], x_d[r0 : r0 + L, :])
                    yt = io.tile([LMAX + 1, W], F32, tag="yt", name="yt")
                    if kind == "first":
                        K = L
                        nc.sync.dma_start(yt[0:K, :], y_d[r0 : r0 + L, :])
                        wy = wyf
                    else:
                        K = L + 1
                        nc.sync.dma_start(yt[0:K, :], y_d[r0 - 1 : r0 + L, :])
                        wy = wyi if kind == "int" else wyl

                    psum = ps.tile([LMAX, W], F32, tag="psb", name="psb")
                    for c in range(NBANK):
                        c0, c1 = c * 512, (c + 1) * 512
                        nc.tensor.matmul(
                            psum[0:L, c0:c1],
                            wy[0:K, 0:L],
                            yt[0:K, c0:c1],
                            start=True,
                            stop=fast_dx,
                        )
                        if not fast_dx:
                            # kx1 * x over [c0, min(c1, W-1))
                            hi = min(c1, W - 1)
                            nc.tensor.matmul(
                                psum[0:L, c0:hi],
                                wx1[0:L, 0:L],
                                xt[0:L, c0:hi],
                                start=False,
                                stop=False,
                            )
                            # kx0 * x[j-1] over [max(c0,1), c1)
                            lo = max(c0, 1)
                            nc.tensor.matmul(
                                psum[0:L, lo:c1],
                                wx0[0:L, 0:L],
                                xt[0:L, lo - 1 : c1 - 1],
                                start=False,
                                stop=True,
                            )

                    ot = io.tile([LMAX, W], F32, tag="ot", name="ot")
                    if fast_dx:
                        if kx == (-1.0, 1.0):
                            in0, in1 = xt[0:L, 1 : W - 1], xt[0:L, 0 : W - 2]
                        else:
                            in0, in1 = xt[0:L, 0 : W - 2], xt[0:L, 1 : W - 1]
                        nc.gpsimd.tensor_tensor(
                            ot[0:L, 1 : W - 1], in0, in1, AluOpType.subtract
                        )
                        nc.scalar.mul(ot[0:L, 0:1], xt[0:L, 0:1], kx[1])
                        nc.scalar.mul(ot[0:L, W - 1 : W], xt[0:L, W - 2 : W - 1], kx[0])
                        nc.vector.tensor_tensor(
                            ot[0:L, :], ot[0:L, :], psum[0:L, :], AluOpType.add
                        )
                    else:
                        nc.vector.tensor_copy(ot[0:L, :], psum[0:L, :])
                    nc.sync.dma_start(out_d[r0 : r0 + L, :], ot[0:L, :])
    nc.compile()
    return nc


_cache = {}


def _get_nc(kx, ky):
    key = (kx, ky)
    if key not in _cache:
        _cache[key] = _build(kx, ky)
    return _cache[key]


def run(x, y, kx, ky, **spmd_kwargs):
    """Run the kernel on full inputs; returns (out [B,C,H,W], BassKernelResults)."""
    assert x.shape == (B, C, H, W) and y.shape == (B, C, H, W)
    kxt = (float(kx[0]), float(kx[1]))
    kyt = (float(ky[0]), float(ky[1]))
    nc = _get_nc(kxt, kyt)
    wts = _weights(kxt, kyt)

    xf = np.ascontiguousarray(x, dtype=np.float32).reshape(B * H, W)
    yf = np.ascontiguousarray(y, dtype=np.float32).reshape(B * H, W)
    in_maps = []
    for i in range(NCORES):
        in_maps.append(
            {
                "x": xf[i * RPC : (i + 1) * RPC],
                "y": yf[i * RPC : (i + 1) * RPC],
                **wts,
            }
        )
    res = run_bass_kernel_spmd(nc, in_maps, list(range(NCORES)), **spmd_kwargs)
    out = np.empty((B * H, W), dtype=np.float32)
    for i, r in enumerate(res.results):
        out[i * RPC : (i + 1) * RPC] = r["out"]
    return out.reshape(B, C, H, W), res


def kernel(x, y, kx, ky):
    return run(np.asarray(x), np.asarray(y), np.asarray(kx), np.asarray(ky))[0]


def bench(x, y, kx, ky, repeat=9, reps=5):
    """Estimate per-execution HW time (ns).

    No NTFF profiling hook is available under this axon build, so this
    builds a second program whose NEFF runs the whole per-core pipeline
    `repeat` times back-to-back, and reports
        (wall(repeat) - wall(1)) / (repeat - 1)
    over device-resident operands -- host/RPC overhead cancels in the
    difference and the repeats measure warm steady-state."""
    import time

    import jax
    from jax.sharding import Mesh, NamedSharding, PartitionSpec
    from jax.experimental.shard_map import shard_map

    from concourse.bass2jax import (
        _bass_exec_p,
        install_neuronx_cc_hook,
        partition_id_tensor,
    )

    install_neuronx_cc_hook()
    kxt = (float(kx[0]), float(kx[1]))
    kyt = (float(ky[0]), float(ky[1]))
    wts = _weights(kxt, kyt)

    devices = jax.devices()[:NCORES]
    mesh = Mesh(np.asarray(devices), ("core",))
    pspec = PartitionSpec("core")
    sharding = NamedSharding(mesh, pspec)

    xf = np.ascontiguousarray(x, dtype=np.float32).reshape(B * H, W)
    yf = np.ascontiguousarray(y, dtype=np.float32).reshape(B * H, W)
    name_to_arr = {
        "x": xf,
        "y": yf,
        **{k: np.concatenate([v] * NCORES, axis=0) for k, v in wts.items()},
    }

    def timed_call(nc):
        partition_name = (
            nc.partition_id_tensor.name if nc.partition_id_tensor else None
        )
        in_names, out_names, out_avals, zero_shapes = [], [], [], []
        for alloc in nc.m.functions[0].allocations:
            if not isinstance(alloc, mybir.MemoryLocationSet):
                continue
            name = alloc.memorylocations[0].name
            if alloc.kind == "ExternalInput":
                if name != partition_name:
                    in_names.append(name)
            elif alloc.kind == "ExternalOutput":
                out_names.append(name)
                shape = tuple(alloc.tensor_shape)
                dtype = mybir.dt.np(alloc.dtype)
                out_avals.append(jax.core.ShapedArray(shape, dtype))
                zero_shapes.append((shape, dtype))
        n_params = len(in_names)
        all_in_names = in_names + out_names + (
            [partition_name] if partition_name else []
        )

        def _body(*args):
            operands = list(args)
            if partition_name is not None:
                operands.append(partition_id_tensor())
            return tuple(
                _bass_exec_p.bind(
                    *operands,
                    out_avals=tuple(out_avals),
                    in_names=tuple(all_in_names),
                    out_names=tuple(out_names),
                    lowering_input_output_aliases=(),
                    sim_require_finite=True,
                    sim_require_nnan=True,
                    nc=nc,
                )
            )

        nin = n_params + len(out_names)
        fn = jax.jit(
            shard_map(
                _body,
                mesh=mesh,
                in_specs=(pspec,) * nin,
                out_specs=(pspec,) * len(out_names),
                check_rep=False,
            ),
            keep_unused=True,
        )
        operands = [jax.device_put(name_to_arr[n], sharding) for n in in_names]
        operands += [
            jax.device_put(np.zeros((NCORES * s[0], *s[1:]), d), sharding)
            for (s, d) in zero_shapes
        ]
        jax.block_until_ready(fn(*operands))  # compile + warm
        times = []
        for _ in range(reps):
            t0 = time.perf_counter()
            jax.block_until_ready(fn(*operands))
            times.append(time.perf_counter() - t0)
        return min(times)

    t1 = timed_call(_get_nc(kxt, kyt))
    key = (kxt, kyt, repeat)
    if key not in _cache:
        _cache[key] = _build(kxt, kyt, repeat=repeat)
    tn = timed_call(_cache[key])
    marginal = (tn - t1) / (repeat - 1)
    print(
        f"bench: wall(x1)={t1 * 1e3:.2f}ms  wall(x{repeat})={tn * 1e3:.2f}ms  "
        f"per-exec={marginal * 1e6:.0f}us"
    )
    return marginal * 1e9


# revision 7
# speedup vs baseline: 5.0338x; 5.0338x over previous
"""Trainium2 Bass kernel for nn_Div_86887188398977.

Computes, per (batch, channel) image with C == 1:
    xp = pad(x[..., :-1], width (1,1));  yp = pad(y[..., :-1, :], height (1,1))
    out = kx0*xp[..., :-1] + kx1*xp[..., 1:] + ky0*yp[..., :-1, :] + ky1*yp[..., 1:, :]
i.e. per element (j = width, i = height):
    out[i, j] = kx0*x[i, j-1]         (j >= 1)
              + kx1*x[i, j]           (j <= W-2)
              + ky0*y[i-1, j]         (i >= 1)
              + ky1*y[i, j]           (i <= H-2)

Sharding: pure data parallel over the batch axis, 16 batches -> 8 cores x 2.

Per-core layout: images flattened to [4096, 2048] rows; row tiles of <=127
output rows with H (rows) on SBUF partitions and W contiguous on the free axis.

Work split per tile (all fp32, numerically exact):
  - dy (the cross-partition height shift) runs on the TensorEngine as 4
    accumulating fp32 matmuls (one per 512-col PSUM bank) with a banded
    lhsT that also encodes the height boundary masks:
       interior tiles: yt = y[r0-1 : r0+L]  (K = L+1), lhsT[m,m] = ky0,
                       lhsT[m+1,m] = ky1
       batch-first:    yt = y[r0 : r0+L]    (K = L),   lhsT[m,m] = ky1,
                       lhsT[m-1,m] = ky0    (row 0 drops the ky0 term)
       batch-last:     interior form with the ky1 coeff of the final row
                       zeroed (row H-1 drops the ky1 term)
  - dx mid columns (1..W-2) on GPSIMD (tensor_tensor, kx = +-1 fast path)
  - dx edge columns 0 / W-1 on ScalarE
  - final out = dx + dy on the VectorEngine (one TT add, PSUM operand)
  - loads on the SP HWDGE ring, stores on the ACT HWDGE ring

For general (non +-1) kx the dx term is emitted as additional fp32 PE
matmuls with scaled-identity weights instead (slower but exact).
"""

import sys

if "/opt/trn_rl_repo" not in sys.path:
    sys.path.insert(0, "/opt/trn_rl_repo")

import numpy as np

import concourse.bacc as bacc
import concourse.mybir as mybir
from concourse.mybir import AluOpType
from concourse.tile import TileContext
from concourse.bass_utils import run_bass_kernel_spmd

B, C, H, W = 16, 1, 2048, 2048
NCORES = 8
BPC = B // NCORES  # batches per core
RPC = BPC * H  # flattened rows per core
F32 = mybir.dt.float32
LMAX = 127
NBANK = W // 512


def _batch_tiles():
    """(r0, L, kind) within one H=2048 image."""
    tiles = []
    r0 = 0
    while r0 < H:
        L = min(LMAX, H - r0)
        kind = "first" if r0 == 0 else ("last" if r0 + L == H else "int")
        tiles.append((r0, L, kind))
        r0 += L
    return tiles


def _weights(kx, ky):
    ky0, ky1 = ky
    kx0, kx1 = kx
    L = LMAX
    last_L = _batch_tiles()[-1][1]
    wy_first = np.zeros((L, L), dtype=np.float32)
    wy_first[np.arange(L), np.arange(L)] = ky1
    wy_first[np.arange(L - 1), np.arange(L - 1) + 1] = ky0
    wy_int = np.zeros((L + 1, L), dtype=np.float32)
    wy_int[np.arange(L), np.arange(L)] = ky0
    wy_int[np.arange(L) + 1, np.arange(L)] = ky1
    wy_last = np.zeros((last_L + 1, last_L), dtype=np.float32)
    wy_last[np.arange(last_L), np.arange(last_L)] = ky0
    wy_last[np.arange(last_L) + 1, np.arange(last_L)] = ky1
    wy_last[last_L, last_L - 1] = 0.0
    wx0 = kx0 * np.eye(L, dtype=np.float32)
    wx1 = kx1 * np.eye(L, dtype=np.float32)
    return {
        "wy_first": wy_first,
        "wy_int": wy_int,
        "wy_last": wy_last,
        "wx0": wx0,
        "wx1": wx1,
    }


def _build(kx, ky, repeat=1):
    """Winning structure (HW-bisected): SWDGE (gpsimd-issued) DMAs for the
    three 1 MB transfers per tile, software-pipelined emission with prefetch
    distance 6 so store-completion waits never head-of-line block upcoming
    load descriptor generation on the single Q7 SWDGE context, xt/yt pools
    8 deep, dx + final add on the VectorEngine."""
    fast_dx = kx in ((-1.0, 1.0), (1.0, -1.0))
    last_L = _batch_tiles()[-1][1]
    DIST = 6

    nc = bacc.Bacc("TRN2", target_bir_lowering=False, debug=False, num_devices=NCORES)
    x_d = nc.declare_dram_parameter("x", [RPC, W], F32, isOutput=False)
    y_d = nc.declare_dram_parameter("y", [RPC, W], F32, isOutput=False)
    wyf_d = nc.declare_dram_parameter("wy_first", [LMAX, LMAX], F32, isOutput=False)
    wyi_d = nc.declare_dram_parameter("wy_int", [LMAX + 1, LMAX], F32, isOutput=False)
    wyl_d = nc.declare_dram_parameter("wy_last", [last_L + 1, last_L], F32, isOutput=False)
    wx0_d = nc.declare_dram_parameter("wx0", [LMAX, LMAX], F32, isOutput=False)
    wx1_d = nc.declare_dram_parameter("wx1", [LMAX, LMAX], F32, isOutput=False)
    out_d = nc.declare_dram_parameter("out", [RPC, W], F32, isOutput=True)

    with TileContext(nc) as tc:
        with (
            tc.tile_pool(name="wpool", bufs=1) as wpool,
            tc.tile_pool(name="io", bufs=4) as io,
            tc.tile_pool(name="ps", bufs=2, space="PSUM") as ps,
        ):
            wyf = wpool.tile([LMAX, LMAX], F32)
            nc.sync.dma_start(wyf[:], wyf_d[:])
            wyi = wpool.tile([LMAX + 1, LMAX], F32)
            nc.sync.dma_start(wyi[:], wyi_d[:])
            wyl = wpool.tile([last_L + 1, last_L], F32)
            nc.sync.dma_start(wyl[:], wyl_d[:])
            wx0 = wpool.tile([LMAX, LMAX], F32)
            nc.sync.dma_start(wx0[:], wx0_d[:])
            wx1 = wpool.tile([LMAX, LMAX], F32)
            nc.sync.dma_start(wx1[:], wx1_d[:])
            dma = nc.gpsimd.dma_start

            tiles = []
            for b in range(repeat * BPC):
                base = (b % BPC) * H
                for r0l, L, kind in _batch_tiles():
                    tiles.append((base + r0l, L, kind))

            loaded = {}

            def load(i):
                r0, L, kind = tiles[i]
                xt = io.tile([LMAX, W], F32, tag="xt", name="xt", bufs=8)
                dma(xt[0:L, :], x_d[r0 : r0 + L, :])
                yt = io.tile([LMAX + 1, W], F32, tag="yt", name="yt", bufs=8)
                if kind == "first":
                    K = L
                    dma(yt[0:K, :], y_d[r0 : r0 + L, :])
                    wy = wyf
                else:
                    K = L + 1
                    dma(yt[0:K, :], y_d[r0 - 1 : r0 + L, :])
                    wy = wyi if kind == "int" else wyl
                loaded[i] = (xt, yt, K, wy)

            for i in range(min(DIST + 1, len(tiles))):
                load(i)
            for i in range(len(tiles)):
                r0, L, kind = tiles[i]
                xt, yt, K, wy = loaded.pop(i)
                psum = ps.tile([LMAX, W], F32, tag="psb", name="psb")
                for c in range(NBANK):
                    c0, c1 = c * 512, (c + 1) * 512
                    nc.tensor.matmul(
                        psum[0:L, c0:c1],
                        wy[0:K, 0:L],
                        yt[0:K, c0:c1],
                        start=True,
                        stop=fast_dx,
                    )
                    if not fast_dx:
                        hi = min(c1, W - 1)
                        nc.tensor.matmul(
                            psum[0:L, c0:hi],
                            wx1[0:L, 0:L],
                            xt[0:L, c0:hi],
                            start=False,
                            stop=False,
                        )
                        lo = max(c0, 1)
                        nc.tensor.matmul(
                            psum[0:L, lo:c1],
                            wx0[0:L, 0:L],
                            xt[0:L, lo - 1 : c1 - 1],
                            start=False,
                            stop=True,
                        )

                ot = io.tile([LMAX, W], F32, tag="ot", name="ot", bufs=4)
                if fast_dx:
                    if kx == (-1.0, 1.0):
                        in0, in1 = xt[0:L, 1 : W - 1], xt[0:L, 0 : W - 2]
                    else:
                        in0, in1 = xt[0:L, 0 : W - 2], xt[0:L, 1 : W - 1]
                    nc.vector.tensor_tensor(
                        ot[0:L, 1 : W - 1], in0, in1, AluOpType.subtract
                    )
                    nc.scalar.mul(ot[0:L, 0:1], xt[0:L, 0:1], kx[1])
                    nc.scalar.mul(ot[0:L, W - 1 : W], xt[0:L, W - 2 : W - 1], kx[0])
                    nc.vector.tensor_tensor(
                        ot[0:L, :], ot[0:L, :], psum[0:L, :], AluOpType.add
                    )
                else:
                    nc.vector.tensor_copy(ot[0:L, :], psum[0:L, :])
                if i + DIST + 1 < len(tiles):
                    load(i + DIST + 1)
                dma(out_d[r0 : r0 + L, :], ot[0:L, :])
    nc.compile()
    return nc


_cache = {}


def _get_nc(kx, ky):
    key = (kx, ky)
    if key not in _cache:
        _cache[key] = _build(kx, ky)
    return _cache[key]


def run(x, y, kx, ky, **spmd_kwargs):
    """Run the kernel on full inputs; returns (out [B,C,H,W], BassKernelResults)."""
    assert x.shape == (B, C, H, W) and y.shape == (B, C, H, W)
    kxt = (float(kx[0]), float(kx[1]))
    kyt = (float(ky[0]), float(ky[1]))
    nc = _get_nc(kxt, kyt)
    wts = _weights(kxt, kyt)

    xf = np.ascontiguousarray(x, dtype=np.float32).reshape(B * H, W)
    yf = np.ascontiguousarray(y, dtype=np.float32).reshape(B * H, W)
    in_maps = []
    for i in range(NCORES):
        in_maps.append(
            {
                "x": xf[i * RPC : (i + 1) * RPC],
                "y": yf[i * RPC : (i + 1) * RPC],
                **wts,
            }
        )
    res = run_bass_kernel_spmd(nc, in_maps, list(range(NCORES)), **spmd_kwargs)
    out = np.empty((B * H, W), dtype=np.float32)
    for i, r in enumerate(res.results):
        out[i * RPC : (i + 1) * RPC] = r["out"]
    return out.reshape(B, C, H, W), res


def kernel(x, y, kx, ky):
    return run(np.asarray(x), np.asarray(y), np.asarray(kx), np.asarray(ky))[0]


def bench(x, y, kx, ky, repeat=9, reps=5):
    """Estimate per-execution HW time (ns).

    No NTFF profiling hook is available under this axon build, so this
    builds a second program whose NEFF runs the whole per-core pipeline
    `repeat` times back-to-back, and reports
        (wall(repeat) - wall(1)) / (repeat - 1)
    over device-resident operands -- host/RPC overhead cancels in the
    difference and the repeats measure warm steady-state."""
    import time

    import jax
    from jax.sharding import Mesh, NamedSharding, PartitionSpec
    from jax.experimental.shard_map import shard_map

    from concourse.bass2jax import (
        _bass_exec_p,
        install_neuronx_cc_hook,
        partition_id_tensor,
    )

    install_neuronx_cc_hook()
    kxt = (float(kx[0]), float(kx[1]))
    kyt = (float(ky[0]), float(ky[1]))
    wts = _weights(kxt, kyt)

    devices = jax.devices()[:NCORES]
    mesh = Mesh(np.asarray(devices), ("core",))
    pspec = PartitionSpec("core")
    sharding = NamedSharding(mesh, pspec)

    xf = np.ascontiguousarray(x, dtype=np.float32).reshape(B * H, W)
    yf = np.ascontiguousarray(y, dtype=np.float32).reshape(B * H, W)
    name_to_arr = {
        "x": xf,
        "y": yf,
        **{k: np.concatenate([v] * NCORES, axis=0) for k, v in wts.items()},
    }

    def timed_call(nc):
        partition_name = (
            nc.partition_id_tensor.name if nc.partition_id_tensor else None
        )
        in_names, out_names, out_avals, zero_shapes = [], [], [], []
        for alloc in nc.m.functions[0].allocations:
            if not isinstance(alloc, mybir.MemoryLocationSet):
                continue
            name = alloc.memorylocations[0].name
            if alloc.kind == "ExternalInput":
                if name != partition_name:
                    in_names.append(name)
            elif alloc.kind == "ExternalOutput":
                out_names.append(name)
                shape = tuple(alloc.tensor_shape)
                dtype = mybir.dt.np(alloc.dtype)
                out_avals.append(jax.core.ShapedArray(shape, dtype))
                zero_shapes.append((shape, dtype))
        n_params = len(in_names)
        all_in_names = in_names + out_names + (
            [partition_name] if partition_name else []
        )

        def _body(*args):
            operands = list(args)
            if partition_name is not None:
                operands.append(partition_id_tensor())
            return tuple(
                _bass_exec_p.bind(
                    *operands,
                    out_avals=tuple(out_avals),
                    in_names=tuple(all_in_names),
                    out_names=tuple(out_names),
                    lowering_input_output_aliases=(),
                    sim_require_finite=True,
                    sim_require_nnan=True,
                    nc=nc,
                )
            )

        nin = n_params + len(out_names)
        fn = jax.jit(
            shard_map(
                _body,
                mesh=mesh,
                in_specs=(pspec,) * nin,
                out_specs=(pspec,) * len(out_names),
                check_rep=False,
            ),
            keep_unused=True,
        )
        operands = [jax.device_put(name_to_arr[n], sharding) for n in in_names]
        operands += [
            jax.device_put(np.zeros((NCORES * s[0], *s[1:]), d), sharding)
            for (s, d) in zero_shapes
        ]
        jax.block_until_ready(fn(*operands))  # compile + warm
        times = []
        for _ in range(reps):
            t0 = time.perf_counter()
            jax.block_until_ready(fn(*operands))
            times.append(time.perf_counter() - t0)
        return min(times)

    t1 = timed_call(_get_nc(kxt, kyt))
    key = (kxt, kyt, repeat)
    if key not in _cache:
        _cache[key] = _build(kxt, kyt, repeat=repeat)
    tn = timed_call(_cache[key])
    marginal = (tn - t1) / (repeat - 1)
    print(
        f"bench: wall(x1)={t1 * 1e3:.2f}ms  wall(x{repeat})={tn * 1e3:.2f}ms  "
        f"per-exec={marginal * 1e6:.0f}us"
    )
    return marginal * 1e9
